# revision 35
# baseline (speedup 1.0000x reference)
"""Trainium2 Bass kernel for nn_DPSpikingDecoder.

Math: the leaky-integrator scan v_t = 0.5*v_{t-1} + x_t, the mean over
channels C, and the differential window pooling are all linear maps over
the time axis:  dp[b, w, f] = sum_{c,t} (K[w, t] / C) * spikes[b, c, t, f]
with K = M_pool @ L_scan a [W=40, T=960] matrix.

The stream is laid out [t, c, f] (time-major), fp16, partition-major, so
each 128-row chunk covers 4 consecutive time steps of all 32 channels and
window w's output row is complete after chunk 6w+5 -- the decoder tail
(transpose + MLP layer 1) runs incrementally DURING the stream instead of
serially after it.  K splits exactly into a chunk-local part (6 distinct
[128, 2] weight tiles: pooling column + carry column, identical for every
window) plus a tiny lower-triangular inter-window carry matrix G applied
as one [v, 5] x [v, 256] matmul per 5-window block (0.5^24 decay makes G
one subdiagonal in practice, but the full triangle is exact and free).

Per 30-chunk tile the PE does 30 fp16 [128,2]x[128,256] matmuls into an
interleaved (dp_loc, P) PSUM block; the epilogue for block b runs while
tile b+1 streams: aligned PSUM->SBUF copy, SWDGE de-interleave DMAs,
carry matmul, add, transpose, fp16 cast, and 10 MLP layer-1 matmuls
(packed 4-wide into PE column groups).  After the last block only the
~3 us softmax/scale chain remains.

Sharding: data-parallel over batch B=8 -> one sample per NeuronCore.
"""

import numpy as np
from contextlib import ExitStack

import concourse.bass as bass
import concourse.bacc as bacc
import concourse.tile as tile
from concourse import mybir
from concourse.bass_utils import run_bass_kernel_spmd

F32 = mybir.dt.float32
F16 = mybir.dt.float16
ADD = mybir.AluOpType.add
MAX = mybir.AluOpType.max

B, C, T, F = 8, 32, 960, 256
L_DP, N_DP = 24, 12
W = T // L_DP            # 40 windows
H = 20                   # hidden dim of the MLP

CH = 128                 # rows per matmul chunk
NCH = C * T // CH        # 240 chunks (4 time steps x 32 channels each)
CPW = L_DP * C // CH     # 6 chunks per window
NB = 10                  # stream blocks
WPB = W // NB            # 4 windows per block (one PE column group each)
CPB = WPB * CPW          # 24 chunks per block


def _host_consts():
    """Local kernel Floc[s], carry decay, and inter-window matrix G."""
    Mrow = np.zeros(L_DP)
    Mrow[N_DP:] = 1.0 / N_DP
    Mrow[:N_DP] = -1.0 / N_DP
    Floc = np.array(
        [sum(Mrow[i] * 0.5 ** (i - s) for i in range(s, L_DP)) for s in range(L_DP)]
    )
    g = sum(Mrow[i] * 0.5 ** i for i in range(L_DP))
    G = np.zeros((W, W))
    for w in range(W):
        for v in range(w):
            G[w, v] = (g / 2) * 0.5 ** (L_DP * (w - 1 - v))
    return Floc, G


NCC = 2062               # packed consts image columns (fp16)


def _host_a2():
    """[CH, 12+128+40] fp16: col 2j = local pooling, 2j+1 = carry (chunk
    pos j); cols 12:140 a 128x128 identity for the final transpose-back;
    cols 140:180 G^T fp16 for the carry matmuls."""
    Floc, G = _host_consts()
    img = np.zeros((CH, 2 * CPW + 128 + W), dtype=np.float64)
    for j in range(CPW):
        for p in range(CH):
            s = 4 * j + p // C
            img[p, 2 * j] = Floc[s] / C
            img[p, 2 * j + 1] = 0.5 ** (L_DP - 1 - s) / C
    img[:, 2 * CPW : 2 * CPW + 128] = np.eye(CH)
    img[0:W, 2 * CPW + 128 :] = G.T
    return np.ascontiguousarray(img.astype(np.float16))


def _host_cimg(W2, b2):
    """Packed small consts, one contiguous [128, 141] DMA image:
    cols 0:40 eye(40) on parts 0:40; 40:80 [W2; b2] on parts 0:21;
    col 80 b1 placeholder (patched in kernel()); 81:101 the 4-col-group
    summing matrix; 101:141 G^T (carry matmul lhsT) on parts 0:40."""
    img = np.zeros((128, 141), dtype=np.float32)
    img[0:W, 0:W] = np.eye(W, dtype=np.float32)
    img[0:H, 40:80] = W2.astype(np.float32)
    img[H, 40:80] = b2.astype(np.float32)
    for j in range(4):
        for i in range(H):
            img[32 * j + i, 81 + i] = 1.0
    _, G = _host_consts()
    img[0:W, 101:141] = G.T.astype(np.float32)
    return img


def _host_packed(W1, cimg):
    """One [128, 2062] fp16 image: a2+eye+gt | cimg (f32 bitcast) | w1r."""
    img = np.zeros((CH, NCC), dtype=np.float16)
    img[:, 0:180] = _host_a2()
    img[:, 180:462] = cimg.view(np.float16)
    w1r = np.ascontiguousarray(
        W1.reshape(W, 2, 128, H).transpose(2, 0, 1, 3).reshape(128, 2 * W * H)
    ).astype(np.float16)
    img[:, 462:2062] = w1r
    return np.ascontiguousarray(img)


def _build_program():
    nc = bacc.Bacc(None)
    x = nc.declare_dram_parameter("x", [CH, NCH, F], F16, isOutput=False)
    cst = nc.declare_dram_parameter("cst", [CH, NCC], F16, isOutput=False)
    y = nc.declare_dram_parameter("y", [W, F], F32, isOutput=True)

    with tile.TileContext(nc) as tc, ExitStack() as ctx:
        consts = ctx.enter_context(tc.tile_pool(name="consts", bufs=1))
        xs = ctx.enter_context(tc.tile_pool(name="xs", bufs=5))
        blk = ctx.enter_context(tc.tile_pool(name="blk", bufs=3))
        work = ctx.enter_context(tc.tile_pool(name="work", bufs=1))
        pdp = ctx.enter_context(tc.tile_pool(name="pdp", bufs=2, space="PSUM"))
        pdpT = ctx.enter_context(tc.tile_pool(name="pdpT", bufs=2, space="PSUM"))
        php = ctx.enter_context(tc.tile_pool(name="php", bufs=1, space="PSUM"))
        ptail = ctx.enter_context(tc.tile_pool(name="ptail", bufs=1, space="PSUM"))

        # one packed consts DMA leads the sync ring while the first x
        # segment leads scalar -- the PE ramp waits on neither for long
        cst_sb = consts.tile([CH, NCC], F16)
        nc.sync.dma_start(out=cst_sb, in_=cst[:])
        a2_sb = cst_sb[:, 0 : 2 * CPW]
        eye16_sb = cst_sb[:, 2 * CPW : 2 * CPW + 128]
        gt16_sb = cst_sb[0:W, 2 * CPW + 128 : 2 * CPW + 128 + W]
        ci_sb = cst_sb[:, 180:462].bitcast(F32)
        w1_sb = cst_sb[:, 462:2062]
        eye_sb = ci_sb[0:W, 0:W]
        w2b_sb = ci_sb[0 : H + 1, 40:80]
        b1_sb = ci_sb[0:H, 80:81]
        sel_sb = ci_sb[:, 81:101]

        # persistent stream-tail state
        P_sb = work.tile([W, F], F16)       # carry rows, de-interleaved
        dpT16 = work.tile([128, 2, W], F16) # dp^T for the MLP contraction
        h_aug = work.tile([H + 1, 1], F32)  # [h; 1]: layer 2 adds b2 in-matmul
        nc.vector.memset(h_aug, 1.0)

        hp_ps = php.tile([128, 1], F32)

        deint_tiles = {}

        def epilogue_pre(w_lo, nw, dpP_t, wl0, hw_ring=False):
            """PSUM drain + de-interleave for windows [w_lo, w_lo+nw):
            DVE + DMA only, no PE ops.  wl0 = first window's row group in
            dpP_t."""
            hi = 32 * (wl0 + nw - 1) + 2
            dpP_sb = blk.tile([32 * (WPB - 1) + 2, F], F32, tag="dpP_sb")
            nc.vector.tensor_copy(dpP_sb[32 * wl0 : hi, :], dpP_t[32 * wl0 : hi, :])
            dpP16 = blk.tile([32 * (WPB - 1) + 2, F], F16, tag="dpP16")
            nc.vector.tensor_copy(dpP16[32 * wl0 : hi, :], dpP_t[32 * wl0 : hi, :])
            # de-interleave (partition-rearranging) on the SWDGE ring so
            # the HWDGE x stream is never blocked; the stream's last
            # blocks ride the HWDGE rings for their lower fixed latency
            eng_a = nc.sync if hw_ring else nc.gpsimd
            eng_b = nc.scalar if hw_ring else nc.gpsimd
            dploc_blk = blk.tile([WPB, F], F32, tag="dploc")
            deint_tiles[w_lo] = dploc_blk
            eng_a.dma_start(
                out=dploc_blk[0:nw, :],
                in_=dpP_sb[32 * wl0 : hi - 1 : 32, :],
            )
            eng_b.dma_start(
                out=P_sb[w_lo : w_lo + nw, :],
                in_=dpP16[32 * wl0 + 1 : hi : 32, :],
            )

        def epilogue(w_lo, nw):
            """PE part: transpose + carry + cast + MLP layer-1 matmuls."""
            dploc_blk = deint_tiles[w_lo]
            # dp^T for these windows: transpose of the local rows, then
            # the inter-window carry accumulated in transposed form:
            #   dpT[f, w] += sum_v P[v, f] * G[w, v]
            dpT_b = pdpT.tile([128, 2, WPB], F32, tag="dpT")
            for e in range(2):
                nc.tensor.matmul(
                    dpT_b[:, e, 0:nw],
                    lhsT=dploc_blk[0:nw, e * 128 : (e + 1) * 128],
                    rhs=eye_sb[0:nw, 0:nw],
                    is_transpose=True,
                    start=True,
                    stop=False,
                    skip_group_check=True,
                )
                nc.tensor.matmul(
                    dpT_b[:, e, 0:nw],
                    lhsT=P_sb[0 : w_lo + nw, e * 128 : (e + 1) * 128],
                    rhs=gt16_sb[0 : w_lo + nw, w_lo : w_lo + nw],
                    start=False,
                    stop=True,
                    skip_group_check=True,
                )
            nc.vector.tensor_copy(dpT16[:, :, w_lo : w_lo + nw], dpT_b[:, :, 0:nw])
            # MLP layer 1, packed 4-wide into PE column groups
            for wl in range(nw):
                for e in range(2):
                    w = w_lo + wl
                    mi = 2 * w + e
                    jg = mi % 4
                    nc.tensor.matmul(
                        hp_ps[32 * jg : 32 * jg + H, :],
                        lhsT=w1_sb[:, mi * H : (mi + 1) * H],
                        rhs=dpT16[:, e, w : w + 1],
                        start=(mi < 4),
                        stop=(mi >= 2 * W - 4),
                        tile_position=(0, 32 * jg),
                        skip_group_check=True,
                    )

        # ---- streamed contraction, tail work folded between tiles ----
        # first/last blocks stream in smaller segments so the PE ramps
        # sooner and the final matmul drain is short
        seg_plan = {0: [6, 6, 12], NB - 1: [6, 6, 6, 6]}
        seg_default = [CPB]
        seg_bufs = {CPB: 6, 12: 2, 6: 4}
        dpP_tiles = {}
        ring = [0]

        def next_eng():
            ring[0] ^= 1
            return nc.scalar if ring[0] else nc.sync

        def chunk_mm(dpP_t, wl, j, xt, sl):
            nc.tensor.matmul(
                dpP_t[32 * wl : 32 * wl + 2, :],
                lhsT=a2_sb[:, 2 * j : 2 * j + 2],
                rhs=xt[:, sl, :],
                start=(j == 0),
                stop=(j == CPW - 1),
                tile_position=(0, 32 * wl),
            )

        for b in range(NB - 1):
            segs = seg_plan.get(b, seg_default)
            seg_tiles = []
            eng = next_eng()
            for i, n in enumerate(segs):
                xt = xs.tile([CH, n, F], F16, tag=f"xt{n}", bufs=seg_bufs[n])
                base = b * CPB + sum(segs[:i])
                eng.dma_start(out=xt, in_=x[:, base : base + n, :])
                seg_tiles += [(xt, s) for s in range(n)]
            if b >= 1:
                epilogue_pre(WPB * (b - 1), WPB, dpP_tiles[b - 1], 0)
            dpP_b = pdp.tile([32 * (WPB - 1) + 2, F], F32, tag="dpP")
            dpP_tiles[b] = dpP_b
            for s in range(CPB):
                wl, j = divmod(s, CPW)
                xt, sl = seg_tiles[s]
                chunk_mm(dpP_b, wl, j, xt, sl)
            if b >= 2:
                epilogue(WPB * (b - 2), WPB)

        # last block: two 2-window PSUM tiles so each half's epilogue can
        # drain while the other half's matmuls still run
        b = NB - 1
        segs = seg_plan[b]
        seg_tiles = []
        for i, n in enumerate(segs):
            xt = xs.tile([CH, n, F], F16, tag=f"xt{n}", bufs=seg_bufs[n])
            base = b * CPB + sum(segs[:i])
            next_eng().dma_start(out=xt, in_=x[:, base : base + n, :])
            seg_tiles += [(xt, s) for s in range(n)]
        epilogue_pre(WPB * (b - 1), WPB, dpP_tiles[b - 1], 0, hw_ring=True)
        epilogue(WPB * (b - 2), WPB)
        dpP_a = pdp.tile([32 * (WPB - 1) + 2, F], F32, tag="dpP")
        dpP_c = pdp.tile([32 * (WPB - 1) + 2, F], F32, tag="dpP")
        for s in range(CPB // 2):
            wl, j = divmod(s, CPW)
            xt, sl = seg_tiles[s]
            chunk_mm(dpP_a, wl, j, xt, sl)
        epilogue_pre(W - 4, 2, dpP_a, 0, hw_ring=True)
        epilogue(WPB * (b - 1), WPB)
        for s in range(CPB // 2, CPB):
            wl, j = divmod(s, CPW)
            xt, sl = seg_tiles[s]
            chunk_mm(dpP_c, wl - 2, j, xt, sl)
        epilogue_pre(W - 2, 2, dpP_c, 0, hw_ring=True)
        epilogue(W - 4, 2)
        epilogue(W - 2, 2)

        # ---- MLP tail: relu, layer 2 + softmax, scale, store ----
        hp_sb = work.tile([128, 1], F32)
        nc.vector.tensor_copy(hp_sb, hp_ps)
        tailf = ptail.tile([W, 42], F32, tag="tailf")
        h_ps = tailf[0:H, 0:1]
        nc.tensor.matmul(h_ps, lhsT=sel_sb, rhs=hp_sb, start=True, stop=True, skip_group_check=True)
        nc.scalar.activation(
            h_aug[0:H, :], h_ps, mybir.ActivationFunctionType.Relu, bias=b1_sb
        )
        a2_ps = tailf[0:1, 1:41]
        nc.tensor.matmul(a2_ps, lhsT=h_aug, rhs=w2b_sb, start=True, stop=True, skip_group_check=True)
        e_sb = work.tile([1, W], F32)
        ssum = work.tile([1, 1], F32)
        nc.scalar.activation(
            e_sb, a2_ps, mybir.ActivationFunctionType.Exp, accum_out=ssum[:]
        )
        rin = work.tile([1, 1], F32)
        nc.vector.reciprocal(rin, ssum)
        ta_sb = work.tile([1, W], F32)
        nc.vector.tensor_scalar_mul(ta_sb, e_sb, rin[:])
        taT_ps = tailf[0:W, 41:42]
        nc.tensor.transpose(taT_ps, ta_sb, ci_sb[0:1, 0:1])
        ta_col = work.tile([W, 1], F32)
        nc.vector.tensor_copy(ta_col, taT_ps)
        # transpose dp^T back to [W, F] and scale by the attention weights
        dpF_ps = ptail.tile([W, 2, 128], F16, tag="dpF")
        for e in range(2):
            nc.tensor.matmul(
                dpF_ps[:, e, :],
                lhsT=dpT16[:, e, :],
                rhs=eye16_sb,
                is_transpose=True,
                start=True,
                stop=True,
                skip_group_check=True,
            )
        att = work.tile([W, F], F32)
        # scale the two halves on different engines so both stores launch
        # at the same time
        nc.vector.tensor_scalar_mul(att[:, 0:128], dpF_ps[:, 0, :], ta_col[:])
        nc.sync.dma_start(out=y[:, 0:128], in_=att[:, 0:128])
        nc.scalar.mul(att[:, 128:256], dpF_ps[:, 1, :], ta_col[:])
        nc.scalar.dma_start(out=y[:, 128:256], in_=att[:, 128:256])

    nc.compile()
    return nc


_CACHED = {}


def _get_program():
    if "nc" not in _CACHED:
        _CACHED["nc"] = _build_program()
    return _CACHED["nc"]


def _in_maps(spikes, W1, b1, W2, b2):
    spikes = np.asarray(spikes, dtype=np.float32)
    W1 = np.asarray(W1, dtype=np.float32)
    b1 = np.asarray(b1, dtype=np.float32)
    W2 = np.asarray(W2, dtype=np.float32)
    b2 = np.asarray(b2, dtype=np.float32)
    _get_program()
    # time-major fp16 stream, partition-major: x[p, m, f] = xt[t(m,p), c(p), f]
    x16 = (
        spikes.astype(np.float16)
        .transpose(0, 2, 1, 3)              # [B, T, C, F]
        .reshape(B, NCH, CH, F)
        .transpose(0, 2, 1, 3)              # [B, 128, 240, F]
    )
    cimg = _host_cimg(W2, b2)
    cimg[0:H, 80] = b1
    shared = {"cst": _host_packed(W1, cimg)}
    return [{"x": np.ascontiguousarray(x16[b]), **shared} for b in range(B)]


def kernel(spikes, W1, b1, W2, b2):
    in_maps = _in_maps(spikes, W1, b1, W2, b2)
    res = run_bass_kernel_spmd(_get_program(), in_maps, list(range(B)))
    out = np.stack([np.asarray(res.results[i]["y"]).reshape(W * F) for i in range(B)])
    return out.astype(np.float32)


# revision 36
# speedup vs baseline: 1.0080x; 1.0080x over previous
"""Trainium2 Bass kernel for nn_DPSpikingDecoder.

Math: the leaky-integrator scan v_t = 0.5*v_{t-1} + x_t, the mean over
channels C, and the differential window pooling are all linear maps over
the time axis:  dp[b, w, f] = sum_{c,t} (K[w, t] / C) * spikes[b, c, t, f]
with K = M_pool @ L_scan a [W=40, T=960] matrix.

The stream is laid out [t, c, f] (time-major), fp16, partition-major, so
each 128-row chunk covers 4 consecutive time steps of all 32 channels and
window w's output row is complete after chunk 6w+5 -- the decoder tail
(transpose + MLP layer 1) runs incrementally DURING the stream instead of
serially after it.  K splits exactly into a chunk-local part (6 distinct
[128, 2] weight tiles: pooling column + carry column, identical for every
window) plus a tiny lower-triangular inter-window carry matrix G applied
as one [v, 5] x [v, 256] matmul per 5-window block (0.5^24 decay makes G
one subdiagonal in practice, but the full triangle is exact and free).

Per 30-chunk tile the PE does 30 fp16 [128,2]x[128,256] matmuls into an
interleaved (dp_loc, P) PSUM block; the epilogue for block b runs while
tile b+1 streams: aligned PSUM->SBUF copy, SWDGE de-interleave DMAs,
carry matmul, add, transpose, fp16 cast, and 10 MLP layer-1 matmuls
(packed 4-wide into PE column groups).  After the last block only the
~3 us softmax/scale chain remains.

Sharding: data-parallel over batch B=8 -> one sample per NeuronCore.
"""

import numpy as np
from contextlib import ExitStack

import concourse.bass as bass
import concourse.bacc as bacc
import concourse.tile as tile
from concourse import mybir
from concourse.bass_utils import run_bass_kernel_spmd

F32 = mybir.dt.float32
F16 = mybir.dt.float16
ADD = mybir.AluOpType.add
MAX = mybir.AluOpType.max

B, C, T, F = 8, 32, 960, 256
L_DP, N_DP = 24, 12
W = T // L_DP            # 40 windows
H = 20                   # hidden dim of the MLP

CH = 128                 # rows per matmul chunk
NCH = C * T // CH        # 240 chunks (4 time steps x 32 channels each)
CPW = L_DP * C // CH     # 6 chunks per window
NB = 10                  # stream blocks
WPB = W // NB            # 4 windows per block (one PE column group each)
CPB = WPB * CPW          # 24 chunks per block


def _host_consts():
    """Local kernel Floc[s], carry decay, and inter-window matrix G."""
    Mrow = np.zeros(L_DP)
    Mrow[N_DP:] = 1.0 / N_DP
    Mrow[:N_DP] = -1.0 / N_DP
    Floc = np.array(
        [sum(Mrow[i] * 0.5 ** (i - s) for i in range(s, L_DP)) for s in range(L_DP)]
    )
    g = sum(Mrow[i] * 0.5 ** i for i in range(L_DP))
    G = np.zeros((W, W))
    for w in range(W):
        for v in range(w):
            G[w, v] = (g / 2) * 0.5 ** (L_DP * (w - 1 - v))
    return Floc, G


NCC = 2062               # packed consts image columns (fp16)


def _host_a2():
    """[CH, 12+128+40] fp16: col 2j = local pooling, 2j+1 = carry (chunk
    pos j); cols 12:140 a 128x128 identity for the final transpose-back;
    cols 140:180 G^T fp16 for the carry matmuls."""
    Floc, G = _host_consts()
    img = np.zeros((CH, 2 * CPW + 128 + W), dtype=np.float64)
    for j in range(CPW):
        for p in range(CH):
            s = 4 * j + p // C
            img[p, 2 * j] = Floc[s] / C
            img[p, 2 * j + 1] = 0.5 ** (L_DP - 1 - s) / C
    img[:, 2 * CPW : 2 * CPW + 128] = np.eye(CH)
    img[0:W, 2 * CPW + 128 :] = G.T
    return np.ascontiguousarray(img.astype(np.float16))


def _host_cimg(W2, b2):
    """Packed small consts, one contiguous [128, 141] DMA image:
    cols 0:40 eye(40) on parts 0:40; 40:80 [W2; b2] on parts 0:21;
    col 80 b1 placeholder (patched in kernel()); 81:101 the 4-col-group
    summing matrix; 101:141 G^T (carry matmul lhsT) on parts 0:40."""
    img = np.zeros((128, 141), dtype=np.float32)
    img[0:W, 0:W] = np.eye(W, dtype=np.float32)
    img[0:H, 40:80] = W2.astype(np.float32)
    img[H, 40:80] = b2.astype(np.float32)
    for j in range(4):
        for i in range(H):
            img[32 * j + i, 81 + i] = 1.0
    _, G = _host_consts()
    img[0:W, 101:141] = G.T.astype(np.float32)
    return img


def _host_packed(W1, cimg):
    """One [128, 2062] fp16 image: a2+eye+gt | cimg (f32 bitcast) | w1r."""
    img = np.zeros((CH, NCC), dtype=np.float16)
    img[:, 0:180] = _host_a2()
    img[:, 180:462] = cimg.view(np.float16)
    w1r = np.ascontiguousarray(
        W1.reshape(W, 2, 128, H).transpose(2, 0, 1, 3).reshape(128, 2 * W * H)
    ).astype(np.float16)
    img[:, 462:2062] = w1r
    return np.ascontiguousarray(img)


def _build_program():
    nc = bacc.Bacc(None)
    x = nc.declare_dram_parameter("x", [CH, NCH, F], F16, isOutput=False)
    cst = nc.declare_dram_parameter("cst", [CH, NCC], F16, isOutput=False)
    y = nc.declare_dram_parameter("y", [W, F], F32, isOutput=True)

    with tile.TileContext(nc) as tc, ExitStack() as ctx:
        consts = ctx.enter_context(tc.tile_pool(name="consts", bufs=1))
        xs = ctx.enter_context(tc.tile_pool(name="xs", bufs=5))
        blk = ctx.enter_context(tc.tile_pool(name="blk", bufs=3))
        work = ctx.enter_context(tc.tile_pool(name="work", bufs=1))
        pdp = ctx.enter_context(tc.tile_pool(name="pdp", bufs=2, space="PSUM"))
        pdpT = ctx.enter_context(tc.tile_pool(name="pdpT", bufs=2, space="PSUM"))
        php = ctx.enter_context(tc.tile_pool(name="php", bufs=1, space="PSUM"))
        ptail = ctx.enter_context(tc.tile_pool(name="ptail", bufs=1, space="PSUM"))

        # one packed consts DMA leads the sync ring while the first x
        # segment leads scalar -- the PE ramp waits on neither for long
        cst_sb = consts.tile([CH, NCC], F16)
        # a2/eye/gt lead the sync ring (first matmul needs only these);
        # the bulky ci/w1 remainder rides the idle SWDGE ring
        nc.sync.dma_start(out=cst_sb[:, 0:180], in_=cst[:, 0:180])
        nc.gpsimd.dma_start(out=cst_sb[:, 180:], in_=cst[:, 180:])
        a2_sb = cst_sb[:, 0 : 2 * CPW]
        eye16_sb = cst_sb[:, 2 * CPW : 2 * CPW + 128]
        gt16_sb = cst_sb[0:W, 2 * CPW + 128 : 2 * CPW + 128 + W]
        ci_sb = cst_sb[:, 180:462].bitcast(F32)
        w1_sb = cst_sb[:, 462:2062]
        eye_sb = ci_sb[0:W, 0:W]
        w2b_sb = ci_sb[0 : H + 1, 40:80]
        b1_sb = ci_sb[0:H, 80:81]
        sel_sb = ci_sb[:, 81:101]

        # persistent stream-tail state
        P_sb = work.tile([W, F], F16)       # carry rows, de-interleaved
        dpT16 = work.tile([128, 2, W], F16) # dp^T for the MLP contraction
        h_aug = work.tile([H + 1, 1], F32)  # [h; 1]: layer 2 adds b2 in-matmul
        nc.vector.memset(h_aug, 1.0)

        hp_ps = php.tile([128, 1], F32)

        deint_tiles = {}

        def epilogue_pre(w_lo, nw, dpP_t, wl0, hw_ring=False):
            """PSUM drain + de-interleave for windows [w_lo, w_lo+nw):
            DVE + DMA only, no PE ops.  wl0 = first window's row group in
            dpP_t."""
            hi = 32 * (wl0 + nw - 1) + 2
            dpP_sb = blk.tile([32 * (WPB - 1) + 2, F], F32, tag="dpP_sb")
            nc.vector.tensor_copy(dpP_sb[32 * wl0 : hi, :], dpP_t[32 * wl0 : hi, :])
            dpP16 = blk.tile([32 * (WPB - 1) + 2, F], F16, tag="dpP16")
            nc.vector.tensor_copy(dpP16[32 * wl0 : hi, :], dpP_t[32 * wl0 : hi, :])
            # de-interleave (partition-rearranging) on the SWDGE ring so
            # the HWDGE x stream is never blocked; the stream's last
            # blocks ride the HWDGE rings for their lower fixed latency
            eng_a = nc.sync if hw_ring else nc.gpsimd
            eng_b = nc.scalar if hw_ring else nc.gpsimd
            dploc_blk = blk.tile([WPB, F], F32, tag="dploc")
            deint_tiles[w_lo] = dploc_blk
            eng_a.dma_start(
                out=dploc_blk[0:nw, :],
                in_=dpP_sb[32 * wl0 : hi - 1 : 32, :],
            )
            eng_b.dma_start(
                out=P_sb[w_lo : w_lo + nw, :],
                in_=dpP16[32 * wl0 + 1 : hi : 32, :],
            )

        def epilogue(w_lo, nw):
            """PE part: transpose + carry + cast + MLP layer-1 matmuls."""
            dploc_blk = deint_tiles[w_lo]
            # dp^T for these windows: transpose of the local rows, then
            # the inter-window carry accumulated in transposed form:
            #   dpT[f, w] += sum_v P[v, f] * G[w, v]
            dpT_b = pdpT.tile([128, 2, WPB], F32, tag="dpT")
            for e in range(2):
                nc.tensor.matmul(
                    dpT_b[:, e, 0:nw],
                    lhsT=dploc_blk[0:nw, e * 128 : (e + 1) * 128],
                    rhs=eye_sb[0:nw, 0:nw],
                    is_transpose=True,
                    start=True,
                    stop=False,
                    skip_group_check=True,
                )
                nc.tensor.matmul(
                    dpT_b[:, e, 0:nw],
                    lhsT=P_sb[0 : w_lo + nw, e * 128 : (e + 1) * 128],
                    rhs=gt16_sb[0 : w_lo + nw, w_lo : w_lo + nw],
                    start=False,
                    stop=True,
                    skip_group_check=True,
                )
            nc.vector.tensor_copy(dpT16[:, :, w_lo : w_lo + nw], dpT_b[:, :, 0:nw])
            # MLP layer 1, packed 4-wide into PE column groups
            for wl in range(nw):
                for e in range(2):
                    w = w_lo + wl
                    mi = 2 * w + e
                    jg = mi % 4
                    nc.tensor.matmul(
                        hp_ps[32 * jg : 32 * jg + H, :],
                        lhsT=w1_sb[:, mi * H : (mi + 1) * H],
                        rhs=dpT16[:, e, w : w + 1],
                        start=(mi < 4),
                        stop=(mi >= 2 * W - 4),
                        tile_position=(0, 32 * jg),
                        skip_group_check=True,
                    )

        # ---- streamed contraction, tail work folded between tiles ----
        # first/last blocks stream in smaller segments so the PE ramps
        # sooner and the final matmul drain is short
        seg_plan = {0: [6, 6, 12], NB - 1: [6, 6, 6, 6]}
        seg_default = [CPB]
        seg_bufs = {CPB: 6, 12: 2, 6: 4}
        dpP_tiles = {}
        ring = [0]

        def next_eng():
            ring[0] ^= 1
            return nc.scalar if ring[0] else nc.sync

        def chunk_mm(dpP_t, wl, j, xt, sl):
            nc.tensor.matmul(
                dpP_t[32 * wl : 32 * wl + 2, :],
                lhsT=a2_sb[:, 2 * j : 2 * j + 2],
                rhs=xt[:, sl, :],
                start=(j == 0),
                stop=(j == CPW - 1),
                tile_position=(0, 32 * wl),
            )

        for b in range(NB - 1):
            segs = seg_plan.get(b, seg_default)
            seg_tiles = []
            eng = next_eng()
            for i, n in enumerate(segs):
                xt = xs.tile([CH, n, F], F16, tag=f"xt{n}", bufs=seg_bufs[n])
                base = b * CPB + sum(segs[:i])
                eng.dma_start(out=xt, in_=x[:, base : base + n, :])
                seg_tiles += [(xt, s) for s in range(n)]
            if b >= 1:
                epilogue_pre(WPB * (b - 1), WPB, dpP_tiles[b - 1], 0)
            dpP_b = pdp.tile([32 * (WPB - 1) + 2, F], F32, tag="dpP")
            dpP_tiles[b] = dpP_b
            for s in range(CPB):
                wl, j = divmod(s, CPW)
                xt, sl = seg_tiles[s]
                chunk_mm(dpP_b, wl, j, xt, sl)
            if b >= 2:
                epilogue(WPB * (b - 2), WPB)

        # last block: two 2-window PSUM tiles so each half's epilogue can
        # drain while the other half's matmuls still run
        b = NB - 1
        segs = seg_plan[b]
        seg_tiles = []
        for i, n in enumerate(segs):
            xt = xs.tile([CH, n, F], F16, tag=f"xt{n}", bufs=seg_bufs[n])
            base = b * CPB + sum(segs[:i])
            next_eng().dma_start(out=xt, in_=x[:, base : base + n, :])
            seg_tiles += [(xt, s) for s in range(n)]
        epilogue_pre(WPB * (b - 1), WPB, dpP_tiles[b - 1], 0, hw_ring=True)
        epilogue(WPB * (b - 2), WPB)
        dpP_a = pdp.tile([32 * (WPB - 1) + 2, F], F32, tag="dpP")
        dpP_c = pdp.tile([32 * (WPB - 1) + 2, F], F32, tag="dpP")
        for s in range(CPB // 2):
            wl, j = divmod(s, CPW)
            xt, sl = seg_tiles[s]
            chunk_mm(dpP_a, wl, j, xt, sl)
        epilogue_pre(W - 4, 2, dpP_a, 0, hw_ring=True)
        epilogue(WPB * (b - 1), WPB)
        for s in range(CPB // 2, CPB):
            wl, j = divmod(s, CPW)
            xt, sl = seg_tiles[s]
            chunk_mm(dpP_c, wl - 2, j, xt, sl)
        epilogue_pre(W - 2, 2, dpP_c, 0, hw_ring=True)
        epilogue(W - 4, 2)
        epilogue(W - 2, 2)

        # ---- MLP tail: relu, layer 2 + softmax, scale, store ----
        hp_sb = work.tile([128, 1], F32)
        nc.vector.tensor_copy(hp_sb, hp_ps)
        tailf = ptail.tile([W, 42], F32, tag="tailf")
        h_ps = tailf[0:H, 0:1]
        nc.tensor.matmul(h_ps, lhsT=sel_sb, rhs=hp_sb, start=True, stop=True, skip_group_check=True)
        nc.scalar.activation(
            h_aug[0:H, :], h_ps, mybir.ActivationFunctionType.Relu, bias=b1_sb
        )
        a2_ps = tailf[0:1, 1:41]
        nc.tensor.matmul(a2_ps, lhsT=h_aug, rhs=w2b_sb, start=True, stop=True, skip_group_check=True)
        e_sb = work.tile([1, W], F32)
        ssum = work.tile([1, 1], F32)
        nc.scalar.activation(
            e_sb, a2_ps, mybir.ActivationFunctionType.Exp, accum_out=ssum[:]
        )
        rin = work.tile([1, 1], F32)
        nc.vector.reciprocal(rin, ssum)
        ta_sb = work.tile([1, W], F32)
        nc.vector.tensor_scalar_mul(ta_sb, e_sb, rin[:])
        taT_ps = tailf[0:W, 41:42]
        nc.tensor.transpose(taT_ps, ta_sb, ci_sb[0:1, 0:1])
        ta_col = work.tile([W, 1], F32)
        nc.vector.tensor_copy(ta_col, taT_ps)
        # transpose dp^T back to [W, F] and scale by the attention weights
        dpF_ps = ptail.tile([W, 2, 128], F16, tag="dpF")
        for e in range(2):
            nc.tensor.matmul(
                dpF_ps[:, e, :],
                lhsT=dpT16[:, e, :],
                rhs=eye16_sb,
                is_transpose=True,
                start=True,
                stop=True,
                skip_group_check=True,
            )
        att = work.tile([W, F], F32)
        # scale the two halves on different engines so both stores launch
        # at the same time
        nc.vector.tensor_scalar_mul(att[:, 0:128], dpF_ps[:, 0, :], ta_col[:])
        nc.sync.dma_start(out=y[:, 0:128], in_=att[:, 0:128])
        nc.scalar.mul(att[:, 128:256], dpF_ps[:, 1, :], ta_col[:])
        nc.scalar.dma_start(out=y[:, 128:256], in_=att[:, 128:256])

    nc.compile()
    return nc


_CACHED = {}


def _get_program():
    if "nc" not in _CACHED:
        _CACHED["nc"] = _build_program()
    return _CACHED["nc"]


def _in_maps(spikes, W1, b1, W2, b2):
    spikes = np.asarray(spikes, dtype=np.float32)
    W1 = np.asarray(W1, dtype=np.float32)
    b1 = np.asarray(b1, dtype=np.float32)
    W2 = np.asarray(W2, dtype=np.float32)
    b2 = np.asarray(b2, dtype=np.float32)
    _get_program()
    # time-major fp16 stream, partition-major: x[p, m, f] = xt[t(m,p), c(p), f]
    x16 = (
        spikes.astype(np.float16)
        .transpose(0, 2, 1, 3)              # [B, T, C, F]
        .reshape(B, NCH, CH, F)
        .transpose(0, 2, 1, 3)              # [B, 128, 240, F]
    )
    cimg = _host_cimg(W2, b2)
    cimg[0:H, 80] = b1
    shared = {"cst": _host_packed(W1, cimg)}
    return [{"x": np.ascontiguousarray(x16[b]), **shared} for b in range(B)]


def kernel(spikes, W1, b1, W2, b2):
    in_maps = _in_maps(spikes, W1, b1, W2, b2)
    res = run_bass_kernel_spmd(_get_program(), in_maps, list(range(B)))
    out = np.stack([np.asarray(res.results[i]["y"]).reshape(W * F) for i in range(B)])
    return out.astype(np.float32)
